# revision 1
# baseline (speedup 1.0000x reference)
"""LocallyConnected1d Bass kernel for 8 trn2 NeuronCores.

Reference computes, per output position w (1024 of them):
    res[b, w, o] = sum_{c,k} xp[b, c, w+k] * weights[w, o, c, k]   (+ reshape & bias)
with B=64, C_in=64, C_out=64, K=9, and xp = x padded by 4 on both sides.

Strategy: shard the 1024 output positions across the 8 cores (128 each) —
weights (the dominant traffic, 151 MB) are split 8 ways and read once.
Per position the contraction (c,k)=576 is split into 5 partition-chunks
(4 x 128 = [2 taps x 64 ch] + 1 x 64 = [tap 8 x 64 ch]) accumulated in PSUM:
    matmul: out[b, o] += lhsT[f, b].T @ rhs[f, o]
lhsT (stationary) comes from an SBUF-resident copy of the core's x window
stored twice (partitions 0-63 = taps shifted +0, 64-127 = shifted +1), so
every chunk's patch AP is a plain contiguous slice. Weights are streamed
chunk-major ([j, p, w*64+o]) so each DMA is one large contiguous slab.

Positions are processed in pairs (t, t+64) on disjoint PE column groups
(tile_position (0,0)/(0,64)) so both matmuls run concurrently and PSUM's
full 128 partitions hold all 128 positions in one pass. Bias (pre-scrambled
to match the reference's flatten/reshape) is added by K=1 ones-matmuls into
PSUM (no replicated-bias DMA). PSUM banks are zeroed up front and all
matmuls run start=False, so accumulation is per-element and independent of
scheduler ordering (start=True would clear has_written for a whole bank,
clobbering sibling position slices).

Inputs are cast to fp16 on the host (measured end-to-end error ~1.3e-4
relative to the output scale; PSUM accumulation stays fp32). Set
DT_MODE = "fp32" for exact-mode fallback (slower: fp32 matmul is 4
cycles/row and doubles the weight traffic).
"""

import numpy as np

B, C, W, O, K, PAD = 64, 64, 1024, 64, 9, 4
NCORES, WLOC = 8, 128
WIN = WLOC + K - 1  # 136 padded-x positions per core
NJ = 5              # contraction chunks per position
DT_MODE = "fp16"    # "fp16" | "fp32"
PAIRED = True       # tile_position col-pairing (v2); False = v1 halves

_cache = {}


def _build_v2(dt_mode):
    import concourse.bacc as bacc
    import concourse.mybir as mybir
    import concourse.tile as tile
    import concourse.bass as bass

    DT = mybir.dt.float16 if dt_mode == "fp16" else mybir.dt.float32
    F32 = mybir.dt.float32

    nc = bacc.Bacc("TRN2", target_bir_lowering=False, debug=False,
                   num_devices=NCORES)
    x_in = nc.dram_tensor("x", [C, WIN * B], DT, kind="ExternalInput")
    w_in = nc.dram_tensor("w", [NJ, 128, WLOC * O], DT, kind="ExternalInput")
    b_in = nc.dram_tensor("bias", [1, WLOC * O], DT, kind="ExternalInput")
    out = nc.dram_tensor("out", [128, 64 * O], F32, kind="ExternalOutput")

    with tile.TileContext(nc) as tc:
        with (
            tc.tile_pool(name="xpool", bufs=1) as xpool,
            tc.tile_pool(name="wpool", bufs=5) as wpool,
            tc.tile_pool(name="bpool", bufs=1) as bpool,
            tc.tile_pool(name="opool", bufs=2) as opool,
            tc.tile_pool(name="psum", bufs=8, space=bass.MemorySpace.PSUM) as ppool,
        ):
            # x window, stored twice: partitions 64+c hold the +1-shifted rows
            x_t = xpool.tile([128, WIN * B], DT)
            nc.sync.dma_start(x_t[0:64, :], x_in[:, :])
            nc.scalar.dma_start(x_t[64:128, 0:(WIN - 1) * B], x_in[:, B:WIN * B])

            bias_t = bpool.tile([1, WLOC * O], DT, name="bias_t")
            nc.scalar.dma_start(bias_t[:], b_in[:, :])
            ones_t = bpool.tile([1, B], DT, name="ones_t")
            nc.vector.memset(ones_t[:], 1.0)

            psums = [
                ppool.tile([128, 512], F32, tag="acc", name=f"acc{g}")
                for g in range(8)
            ]
            # Zero bank values; all matmuls run start=False and accumulate
            # per-element regardless of scheduling order (start=True would
            # clear has_written for the WHOLE bank, clobbering sibling
            # position slices).
            for g in range(8):
                nc.vector.memset(psums[g][:], 0.0)
                # bias rows: partitions 0-63 get positions 8g..8g+8,
                # partitions 64-127 get positions 64+8g..64+8g+8
                nc.tensor.matmul(
                    psums[g][0:64, :], ones_t[:], bias_t[:, g * 512:(g + 1) * 512],
                    start=False, stop=False, tile_position=(0, 0),
                )
                nc.tensor.matmul(
                    psums[g][64:128, :], ones_t[:],
                    bias_t[:, 4096 + g * 512:4096 + (g + 1) * 512],
                    start=False, stop=False, tile_position=(0, 64),
                )

            for j in range(NJ):
                rows = 128 if j < 4 else 64
                w_t = wpool.tile([128, WLOC * O], DT, tag="w", name=f"w{j}")
                # Column-split per slab across both HWDGE rings: the A-group
                # (positions 0-63) matmuls only depend on the first half, so
                # they start after ~1MB instead of the full 2MB slab.
                half = WLOC * O // 2
                nc.sync.dma_start(w_t[0:rows, 0:half], w_in[j, 0:rows, 0:half])
                nc.scalar.dma_start(
                    w_t[0:rows, half:WLOC * O], w_in[j, 0:rows, half:WLOC * O])
                for t in range(64):
                    sl = slice((t % 8) * O, (t % 8 + 1) * O)
                    offa = (t + 2 * j) * B
                    offb = (t + 64 + 2 * j) * B
                    nc.tensor.matmul(
                        psums[t // 8][0:64, sl],
                        x_t[0:rows, offa:offa + B],
                        w_t[0:rows, t * O:(t + 1) * O],
                        start=False, stop=(j == NJ - 1), tile_position=(0, 0),
                    )
                    nc.tensor.matmul(
                        psums[t // 8][64:128, sl],
                        x_t[0:rows, offb:offb + B],
                        w_t[0:rows, (t + 64) * O:(t + 65) * O],
                        start=False, stop=(j == NJ - 1), tile_position=(0, 64),
                    )

            # Drain bank-by-bank so copies + output DMA overlap the last
            # matmuls instead of forming a serial tail.
            stage = opool.tile([128, 64 * O], F32, name="stage")
            for g in range(8):
                nc.vector.tensor_copy(stage[:, g * 512:(g + 1) * 512], psums[g][:])
                if g == 3:
                    nc.scalar.dma_start(out[:, 0:2048], stage[:, 0:2048])
            nc.sync.dma_start(out[:, 2048:4096], stage[:, 2048:4096])

    nc.compile()
    return nc


def _get_nc():
    key = (DT_MODE, PAIRED)
    if key not in _cache:
        _cache[key] = _build_v2(DT_MODE)
    return _cache[key]


def _prep_inputs(x, weights, bias, dt_np):
    """Build the per-core input maps (host-side shard + layout transform)."""
    xp = np.pad(np.asarray(x, np.float32), ((0, 0), (0, 0), (PAD, PAD)))
    bias_re = np.asarray(bias, np.float32).reshape(W, O)  # flat -> [w, o]
    weights = np.asarray(weights, np.float32)

    in_maps = []
    for r in range(NCORES):
        wb = r * WLOC
        xh = np.ascontiguousarray(
            xp[:, :, wb:wb + WIN].transpose(1, 2, 0)
        ).astype(dt_np).reshape(C, WIN * B)

        # [w, f=(k*64+c), o]
        wt = weights[wb:wb + WLOC].transpose(0, 3, 2, 1).reshape(WLOC, K * C, O)
        wslab = np.zeros((NJ, 128, WLOC * O), dt_np)
        for j in range(NJ):
            rows = 128 if j < 4 else 64
            blk = wt[:, 128 * j:128 * j + rows, :]          # (128 w, rows, O)
            wslab[j, :rows] = (
                blk.transpose(1, 0, 2).reshape(rows, WLOC * O).astype(dt_np)
            )

        # bias rows in pair order: [t-group A (w=0..63) | t-group B (w=64..127)]
        bh = bias_re[wb:wb + WLOC].reshape(1, WLOC * O).astype(dt_np)

        in_maps.append({"x": xh, "w": wslab, "bias": bh})
    return in_maps


def _run(in_maps, **kwargs):
    import concourse.bass_utils as bass_utils

    nc = _get_nc()
    return bass_utils.run_bass_kernel_spmd(
        nc, in_maps, core_ids=list(range(NCORES)), **kwargs
    )


def kernel(x, weights, bias, _extra=None, **run_kwargs):
    dt_np = np.float16 if DT_MODE == "fp16" else np.float32
    in_maps = _prep_inputs(x, weights, bias, dt_np)
    res = _run(in_maps, **run_kwargs)
    # out rows: p = wgrp*64 + b, cols t*64+o  ->  res[b, wb + wgrp*64+t, o]
    parts = []
    for r in range(NCORES):
        o = res.results[r]["out"].reshape(2, 64, 64, O)     # (wgrp, b, t, o)
        parts.append(o.transpose(1, 0, 2, 3).reshape(B, WLOC * O))
    full = np.concatenate(parts, axis=1)                    # (B, W*O), w-major
    result = full.reshape(B, 64, 1024)                      # reference reshape
    if run_kwargs:
        return result, res
    return result



# revision 2
# speedup vs baseline: 1.8139x; 1.8139x over previous
"""LocallyConnected1d Bass kernel for 8 trn2 NeuronCores (v2, fp8).

Reference computes, per output position w (1024 of them):
    res[b, w, o] = sum_{c,k} xp[b, c, w+k] * weights[w, o, c, k]   (+ reshape & bias)
with B=64, C_in=64, C_out=64, K=9, and xp = x padded by 4 on both sides.

Strategy: shard the 1024 output positions across the 8 cores (128 each).
Per position the contraction (c,k)=576 splits into 5 partition-chunks
(4 x 128 = [2 taps x 64 ch] + 1 x 64 = [tap 8 x 64 ch]) accumulated in PSUM:
    matmul: out[b, o] += lhsT[f, b].T @ rhs[f, o]
lhsT comes from an SBUF-resident copy of the core's x window stored twice
(partitions 0-63 = taps shifted +0, 64-127 = shifted +1) so every chunk's
patch AP is a plain contiguous slice.

v2 changes vs the fp16 baseline (50.7us):
 - x and weights are cast to float8_e3m4 on the host (measured end-to-end
   max rel err ~9.6e-3 vs the 2e-2 gate; e3m4 products are exact in fp32
   PSUM accumulation, verified bit-exact on HW).  This halves the dominant
   DMA traffic; the kernel is DMA-bound at ~360 GB/s aggregate.
 - Output staged to fp16 (halves output traffic); bias is added on the host.
 - Positions pair as (t, t+1) in PSUM partitions (0-63 / 64-127); weights
   arrive in 8 bank-sized groups (8 pairs each) so each PSUM bank drains as
   soon as its group's matmuls finish, overlapping the remaining compute.
 - PSUM banks are zeroed up front (DVE) and all matmuls run start=False.
 - A run of zero-valued warmup matmuls keeps the PE busy from ~1us so the
   p-state ramp (cost model: full clock only after 3us of continuous PE
   activity) is paid before the real matmuls begin.
 - Drains (PSUM fp32 -> SBUF fp16) run on the Activation engine; output
   DMAs ride the scalar queue after the x loads; weight DMAs own the sync
   queue.
"""

import numpy as np

B, C, W, O, K, PAD = 64, 64, 1024, 64, 9, 4
NCORES, WLOC = 8, 128
WIN = WLOC + K - 1  # 136 padded-x positions per core
NJ = 5              # contraction chunks per position
NGRP = 8            # weight groups == PSUM banks, 8 position-pairs each
N_WARM = 64         # zero-matmul PE warmup instructions
XSPLIT = 3584       # x piece A covers groups 0-2 (positions < 56)
DT_MODE = "fp8"     # informational; test.py reads this

_cache = {}


def _build(n_warm=N_WARM):
    import concourse.bacc as bacc
    import concourse.mybir as mybir
    import concourse.tile as tile
    import concourse.bass as bass

    DT = mybir.dt.float8e3
    F16 = mybir.dt.float16
    F32 = mybir.dt.float32

    nc = bacc.Bacc("TRN2", target_bir_lowering=False, debug=False,
                   num_devices=NCORES)
    x_in = nc.dram_tensor("x", [C, WIN * B], DT, kind="ExternalInput")
    w_in = nc.dram_tensor("w", [NGRP, 128, 5120], DT, kind="ExternalInput")
    out = nc.dram_tensor("out", [128, 64 * O], F16, kind="ExternalOutput")

    with tile.TileContext(nc) as tc:
        with (
            tc.tile_pool(name="xpool", bufs=1) as xpool,
            tc.tile_pool(name="wpool", bufs=1) as wpool,
            tc.tile_pool(name="opool", bufs=1) as opool,
            tc.tile_pool(name="psum", bufs=8, space=bass.MemorySpace.PSUM) as ppool,
        ):
            z = xpool.tile([128, 64], DT, name="z")
            nc.vector.memset(z[:], 0.0)

            psums = [
                ppool.tile([128, 512], F32, tag="acc", name=f"acc{g}")
                for g in range(NGRP)
            ]
            for g in range(NGRP):
                nc.vector.memset(psums[g][:], 0.0)

            # Warmup: accumulate exact zeros into bank 0 so the PE's p-state
            # ramp elapses before the first real matmul dispatches.
            for i in range(n_warm):
                nc.tensor.matmul(psums[0][0:64, 0:64], z[:], z[:],
                                 start=False, stop=False, tile_position=(0, 0))

            # x window, stored twice: partitions 64+c hold the +1-shifted rows.
            # Piece A covers groups 0-2 so group 0 matmuls start early; piece B
            # streams behind the first weight group.
            x_t = xpool.tile([128, WIN * B], DT, name="x_t")
            nc.scalar.dma_start(x_t[0:64, 0:XSPLIT], x_in[:, 0:XSPLIT])
            nc.scalar.dma_start(x_t[64:128, 0:XSPLIT], x_in[:, B:XSPLIT + B])
            wts = [wpool.tile([128, 5120], DT, name=f"w{g}") for g in range(NGRP)]
            nc.sync.dma_start(wts[0][0:128, 0:4096], w_in[0, 0:128, 0:4096])
            nc.sync.dma_start(wts[0][0:64, 4096:5120], w_in[0, 0:64, 4096:5120])
            nc.scalar.dma_start(x_t[0:64, XSPLIT:WIN * B],
                                x_in[:, XSPLIT:WIN * B])
            nc.scalar.dma_start(x_t[64:128, XSPLIT:(WIN - 1) * B],
                                x_in[:, XSPLIT + B:WIN * B])
            for g in range(1, NGRP):
                nc.sync.dma_start(wts[g][0:128, 0:4096], w_in[g, 0:128, 0:4096])
                nc.sync.dma_start(wts[g][0:64, 4096:5120],
                                  w_in[g, 0:64, 4096:5120])

            stage = opool.tile([128, 64 * O], F16, name="stage")

            for g in range(NGRP):
                w_t = wts[g]
                for pp in range(8):
                    q = 8 * g + pp        # pair index; positions 2q, 2q+1
                    sl = slice(pp * O, (pp + 1) * O)
                    for j in range(NJ):
                        rows = 128 if j < 4 else 64
                        if j < 4:
                            ca = ((pp * 2 + 0) * 4 + j) * O
                            cb = ((pp * 2 + 1) * 4 + j) * O
                        else:
                            ca = 4096 + (pp * 2 + 0) * O
                            cb = 4096 + (pp * 2 + 1) * O
                        offa = (2 * q + 2 * j) * B
                        offb = (2 * q + 1 + 2 * j) * B
                        nc.tensor.matmul(
                            psums[g][0:64, sl],
                            x_t[0:rows, offa:offa + B],
                            w_t[0:rows, ca:ca + O],
                            start=False, stop=(j == NJ - 1),
                            tile_position=(0, 0),
                        )
                        nc.tensor.matmul(
                            psums[g][64:128, sl],
                            x_t[0:rows, offb:offb + B],
                            w_t[0:rows, cb:cb + O],
                            start=False, stop=(j == NJ - 1),
                            tile_position=(0, 64),
                        )
                # Drain this bank (fp32 -> fp16) on the Activation engine so
                # it overlaps the next group's matmuls.
                nc.scalar.copy(stage[:, g * 512:(g + 1) * 512], psums[g][:])
                if g == 5:
                    nc.scalar.dma_start(out[:, 0:3072], stage[:, 0:3072])
                elif g == 6:
                    nc.scalar.dma_start(out[:, 3072:3584], stage[:, 3072:3584])
                elif g == 7:
                    nc.scalar.dma_start(out[:, 3584:4096], stage[:, 3584:4096])

    nc.compile()
    return nc


def _get_nc():
    key = ("v2", N_WARM)
    if key not in _cache:
        _cache[key] = _build(N_WARM)
    return _cache[key]


def _prep_inputs(x, weights, bias=None, dt_np=None):
    """Per-core input maps (host-side shard + fp8 layout transform)."""
    import ml_dtypes

    DT = ml_dtypes.float8_e3m4
    xp = np.pad(np.asarray(x, np.float32), ((0, 0), (0, 0), (PAD, PAD)))
    weights = np.asarray(weights, np.float32)

    in_maps = []
    for r in range(NCORES):
        wb = r * WLOC
        xh = np.ascontiguousarray(
            xp[:, :, wb:wb + WIN].transpose(1, 2, 0)
        ).astype(DT).reshape(C, WIN * B)

        # [pos, f=(k*64+c), o]
        wt = weights[wb:wb + WLOC].transpose(0, 3, 2, 1).reshape(WLOC, K * C, O)
        main = wt[:, :512, :].reshape(NGRP, 8, 2, 4, 128, O)   # g,pp,s,j,f,o
        main = main.transpose(0, 4, 1, 2, 3, 5).reshape(NGRP, 128, 4096)
        tail = wt[:, 512:, :].reshape(NGRP, 8, 2, 64, O)       # g,pp,s,f,o
        tail = tail.transpose(0, 3, 1, 2, 4).reshape(NGRP, 64, 1024)
        wslab = np.zeros((NGRP, 128, 5120), DT)
        wslab[:, :, :4096] = main.astype(DT)
        wslab[:, :64, 4096:] = tail.astype(DT)

        in_maps.append({"x": xh, "w": wslab})
    return in_maps


def _run(in_maps, **kwargs):
    import concourse.bass_utils as bass_utils

    nc = _get_nc()
    return bass_utils.run_bass_kernel_spmd(
        nc, in_maps, core_ids=list(range(NCORES)), **kwargs
    )


def kernel(x, weights, bias, _extra=None, **run_kwargs):
    in_maps = _prep_inputs(x, weights)
    res = _run(in_maps, **run_kwargs)
    parts = []
    for r in range(NCORES):
        o = res.results[r]["out"].astype(np.float32)
        o = o.reshape(2, 64, NGRP, 8, O)          # s, b, g, pp, o
        o = o.transpose(1, 2, 3, 0, 4).reshape(B, WLOC, O)
        parts.append(o)
    full = np.concatenate(parts, axis=1)          # (B, 1024, 64)
    result = full.reshape(B, 64, 1024)            # reference flatten order
    result = result + np.asarray(bias, np.float32)[None, :, :]
    if run_kwargs:
        return result, res
    return result
